# revision 15
# baseline (speedup 1.0000x reference)
"""MoE kernel for TRN2, 8 NeuronCores, data-parallel over the batch dim.

Reference computation (B=8192, D=1024, H=1024, E=16):
    weights = softmax(x @ Wg + bg, axis=1)            # [B, E]
    h       = relu(einsum('bd,edh->beh', x, W1) + b1) # [B, E, H]
    eo      = einsum('beh,eh->be', h, W2) + b2        # [B, E]
    out     = sum(eo * weights, axis=1, keepdims=True)# [B, 1]

Strategy (v2 — bf16 matmuls, col-tiled stage 2, transposed combine):
  - Shard B over 8 cores (1024 rows/core); weights replicated.
  - All heavy matmuls in bf16 (1 cycle/row on PE + fast weight load); the
    d-contraction runs in fp32 PSUM so accuracy stays ~0.3%.
  - Stage 1 per t=(e, h_tile): psum[h=128, b=512] x2 accumulated over 8
    d-tiles from resident xT tiles; ReLU+b1 via ScalarE -> hr bf16.
  - Stage 2: w2 column blocks, 4 PSUM col-groups (partitions 32j..32j+15,
    j=t%4): batches of 4 matmuls on distinct col-groups run concurrently
    in the PE array (~4x cheaper than serial); groups are summed later via
    a replicated-weight reduction.
  - Gating stays transposed end-to-end: logits^T [128, B] with gate
    weights replicated into all 4 col-groups (pad cols zero); softmax is
    exp on ScalarE (bias=bg, pad rows -40 -> 0) + one PE reduction with a
    0.25-weighted ones vector (each expert appears 4x); no transposes.
  - Combine: v = eo_psum * expw (DVE); num = ones^T @ v + (b2/4)^T @ expw;
    y = num * reciprocal(sumexp); y^T DMA'd out as a [1, B] row.
"""

import numpy as np
import ml_dtypes

import concourse.bacc as bacc
import concourse.bass as bass
import concourse.mybir as mybir
from concourse import tile
from concourse.bass_utils import run_bass_kernel_spmd

B, D, H, E = 8192, 1024, 1024, 16
N_CORES = 8
BS = B // N_CORES  # 1024 batch rows per core
BH = 512           # psum-bank-sized half of the batch
DT = D // 128      # 8 d-tiles
HT = H // 128      # 8 h-tiles
T = E * HT         # 128 (e, h_tile) pairs
GB = 4             # stage-2 col-groups

F32 = mybir.dt.float32
F32R = mybir.dt.float32r
BF16 = mybir.dt.bfloat16
AF = mybir.ActivationFunctionType


def build_bass():
    nc = bacc.Bacc("TRN2", target_bir_lowering=False, debug=False)
    xt_d = nc.dram_tensor("xt", [D, BS], BF16, kind="ExternalInput")
    w1_d = nc.dram_tensor("w1p", [T, 128, DT * 128], BF16, kind="ExternalInput")
    b1t_d = nc.dram_tensor("b1t", [128, T], F32, kind="ExternalInput")
    w2bd_d = nc.dram_tensor("w2bd", [128, T * E], BF16, kind="ExternalInput")
    wg4_d = nc.dram_tensor("wg4", [128, DT * 128], BF16, kind="ExternalInput")
    bg4_d = nc.dram_tensor("bg4", [128, 1], F32, kind="ExternalInput")
    b2q4_d = nc.dram_tensor("b2q4", [128, 1], BF16, kind="ExternalInput")
    ones1_d = nc.dram_tensor("ones1", [128, 1], BF16, kind="ExternalInput")
    o025_d = nc.dram_tensor("o025", [128, 1], BF16, kind="ExternalInput")
    y_d = nc.dram_tensor("y", [1, BS], F32, kind="ExternalOutput")

    with tile.TileContext(nc) as tc:
        with (
            tc.tile_pool(name="const", bufs=1) as cpool,
            tc.tile_pool(name="w1", bufs=4) as w1pool,
            tc.tile_pool(name="hrelu", bufs=18) as hpool,
            tc.tile_pool(name="misc", bufs=1) as mpool,
            tc.tile_pool(name="ps1", bufs=2, space=bass.MemorySpace.PSUM) as psh,
            tc.tile_pool(name="ps_eo", bufs=1, space=bass.MemorySpace.PSUM) as pseo,
            tc.tile_pool(name="ps_aux", bufs=2, space=bass.MemorySpace.PSUM) as psaux,
        ):
            # ---- resident tensors ----
            # DMAs are queued per issuing engine: xt + consts ride the sync
            # queue, gate/W1/W2 weights ride the gpsimd queue so the first
            # weight tiles land while xt is still streaming.
            wg4_sb = cpool.tile([128, DT * 128], BF16, tag="wg4")
            nc.gpsimd.dma_start(wg4_sb[:], wg4_d[:])
            xt_sb = []
            for d in range(DT):
                tl = cpool.tile([128, BS], BF16, tag=f"xt{d}")
                nc.sync.dma_start(tl[:], xt_d[d * 128:(d + 1) * 128, :])
                xt_sb.append(tl)
            bg4_sb = cpool.tile([128, 1], F32, tag="bg4")
            nc.sync.dma_start(bg4_sb[:], bg4_d[:])
            b1t_sb = cpool.tile([128, T], F32, tag="b1t")
            nc.sync.dma_start(b1t_sb[:], b1t_d[:])
            b2q4_sb = cpool.tile([128, 1], BF16, tag="b2q4")
            nc.sync.dma_start(b2q4_sb[:], b2q4_d[:])
            ones1_sb = cpool.tile([128, 1], BF16, tag="ones1")
            nc.sync.dma_start(ones1_sb[:], ones1_d[:])
            o025_sb = cpool.tile([128, 1], BF16, tag="o025")
            nc.sync.dma_start(o025_sb[:], o025_d[:])
            w2bd_sb = cpool.tile([128, T * E], BF16, tag="w2bd")

            expw_sb = mpool.tile([128, BS], BF16, tag="expw")
            v_sb = mpool.tile([128, BS], BF16, tag="v")
            serec_sb = mpool.tile([1, BS], F32, tag="serec")
            y_sb = mpool.tile([1, BS], F32, tag="ysb")

            # ---- gating: logits^T, replicated into the 4 col-groups ----
            glog = psaux.tile([128, BS], F32, tag="aux")
            for d in range(DT):
                for bh in range(2):
                    nc.tensor.matmul(
                        glog[:, bh * BH:(bh + 1) * BH],
                        wg4_sb[:, d * 128:(d + 1) * 128],
                        xt_sb[d][:, bh * BH:(bh + 1) * BH],
                        start=(d == 0), stop=(d == DT - 1),
                        skip_group_check=True,
                    )
            # expw = exp(logits + bg); pad rows get bias -40 -> ~0
            nc.scalar.activation(expw_sb[:], glog[:], AF.Exp, bias=bg4_sb[:])

            # ---- stage-2 accumulator; zero pad rows once ----
            eo_ps = pseo.tile([128, BS], F32)
            nc.vector.memset(eo_ps[:], 0.0)

            hrs = {}

            def flush(ts, after_bh=None):
                for bh in range(2):
                    for tt in ts:
                        g = tt % GB
                        nc.tensor.matmul(
                            eo_ps[32 * g:32 * g + 16, bh * BH:(bh + 1) * BH],
                            w2bd_sb[:, tt * E:(tt + 1) * E],
                            hrs[tt][:, bh * BH:(bh + 1) * BH],
                            start=(tt < GB), stop=(tt >= T - GB),
                            skip_group_check=True,
                            tile_position=(0, 32 * g),
                        )
                    if after_bh is not None:
                        after_bh(bh)
                for tt in ts:
                    del hrs[tt]

            # ---- main loop over t=(e, h_tile) ----
            # Stage-2 is flushed in 8-t batches, one full batch behind, so
            # the PE never waits on a freshly produced ReLU tile and the
            # full-array <-> col-tiled pipeline bubble is paid 16x, not 32x.
            FB = 2 * GB
            for t in range(T):
                if t % FB == 0 and t >= 2 * FB:
                    flush(range(t - 2 * FB, t - FB))
                w1t = w1pool.tile([128, DT * 128], BF16, tag="w1t")
                nc.gpsimd.dma_start(w1t[:], w1_d[t, :, :])
                if t == 2:
                    nc.gpsimd.dma_start(w2bd_sb[:], w2bd_d[:])
                hr = hpool.tile([128, BS], BF16, tag="hr")
                for bh in range(2):
                    ps1 = psh.tile([128, BH], F32, tag="ps1")
                    for d in range(DT):
                        nc.tensor.matmul(
                            ps1[:],
                            w1t[:, d * 128:(d + 1) * 128],
                            xt_sb[d][:, bh * BH:(bh + 1) * BH],
                            start=(d == 0), stop=(d == DT - 1),
                            skip_group_check=True,
                        )
                    nc.scalar.activation(
                        hr[:, bh * BH:(bh + 1) * BH], ps1[:], AF.Relu,
                        bias=b1t_sb[:, t:t + 1],
                    )
                hrs[t] = hr
                if t == 2:
                    # sum of gate weights (each expert appears 4x -> 0.25)
                    sumexp = psaux.tile([1, BS], F32, tag="aux")
                    for bh in range(2):
                        nc.tensor.matmul(
                            sumexp[:, bh * BH:(bh + 1) * BH],
                            o025_sb[:], expw_sb[:, bh * BH:(bh + 1) * BH],
                            start=True, stop=True, skip_group_check=True,
                        )
                    nc.vector.reciprocal(serec_sb[:], sumexp[:])
            flush(range(T - 2 * FB, T - FB))

            # ---- combine: y = (1^T(eo*expw) + (b2/4)^T expw) / sumexp ----
            # v for each batch half is computed on DVE as soon as that half's
            # last stage-2 matmul retires, overlapping the other half.
            def emit_v(bh):
                sl = slice(bh * BH, (bh + 1) * BH)
                nc.vector.tensor_mul(v_sb[:, sl], eo_ps[:, sl], expw_sb[:, sl])

            flush(range(T - FB, T), after_bh=emit_v)
            num = psaux.tile([1, BS], F32, tag="aux")
            for bh in range(2):
                sl = slice(bh * BH, (bh + 1) * BH)
                nc.tensor.matmul(
                    num[:, sl], ones1_sb[:], v_sb[:, sl],
                    start=True, stop=False, skip_group_check=True,
                )
                nc.tensor.matmul(
                    num[:, sl], b2q4_sb[:], expw_sb[:, sl],
                    start=False, stop=True, skip_group_check=True,
                )
            nc.vector.tensor_mul(y_sb[:], num[:], serec_sb[:])
            nc.sync.dma_start(y_d[:], y_sb[:])
    nc.compile()
    return nc


def prep_inputs(x, W1, b1, W2, b2, Wg, bg):
    """Host-side data prep. Returns (shared_map, per_core_xt)."""
    f = np.float32
    bf = ml_dtypes.bfloat16
    # W1 [E, D, H] -> [t=(e,ht), d_in, (d_t, h_in)]: per t one contiguous
    # block whose SBUF layout is [128 d_in, 8 d_t * 128 h]
    w1p = np.ascontiguousarray(
        np.asarray(W1, f).reshape(E, DT, 128, HT, 128)
        .transpose(0, 3, 2, 1, 4).reshape(T, 128, DT * 128)).astype(bf)
    b1t = np.ascontiguousarray(
        np.asarray(b1, f).reshape(E, HT, 128).transpose(2, 0, 1).reshape(128, T))
    w2bd = np.zeros((128, T, E), dtype=f)
    for t in range(T):
        e, ht = divmod(t, HT)
        w2bd[:, t, e] = W2[e, ht * 128:(ht + 1) * 128]
    w2bd = w2bd.reshape(128, T * E).astype(bf)
    # gate weights replicated into the 4 col-groups (16 used + 16 pad cols)
    wgr = np.asarray(Wg, f).reshape(DT, 128, E)
    wg4 = np.zeros((DT, 128, 128), dtype=f)
    for j in range(GB):
        wg4[:, :, 32 * j:32 * j + E] = wgr
    wg4 = np.ascontiguousarray(wg4.transpose(1, 0, 2).reshape(128, DT * 128)).astype(bf)
    lane = np.arange(128) % 32
    real = lane < E
    bg4 = np.full((128, 1), -40.0, f)
    bg4[real, 0] = np.tile(np.asarray(bg, f), GB)
    b2q4 = np.zeros((128, 1), f)
    b2q4[real, 0] = np.tile(np.asarray(b2, f) / 4.0, GB)
    b2q4 = b2q4.astype(bf)
    ones1 = np.where(real, 1.0, 0.0).astype(bf).reshape(128, 1)
    o025 = np.where(real, 0.25, 0.0).astype(bf).reshape(128, 1)
    shared = {"w1p": w1p, "b1t": b1t, "w2bd": w2bd, "wg4": wg4, "bg4": bg4,
              "b2q4": b2q4, "ones1": ones1, "o025": o025}
    xT = np.asarray(x, f).T.astype(bf)  # [D, B]
    xts = [np.ascontiguousarray(xT[:, c * BS:(c + 1) * BS]) for c in range(N_CORES)]
    return shared, xts


def run(inputs, trace=False):
    nc = build_bass()
    shared, xts = prep_inputs(**inputs)
    in_maps = [dict(shared, xt=xts[c]) for c in range(N_CORES)]
    res = run_bass_kernel_spmd(
        nc, in_maps, core_ids=list(range(N_CORES)), trace=trace
    )
    y = np.concatenate([r["y"] for r in res.results], axis=1)  # [1, B]
    return np.ascontiguousarray(y.reshape(B, 1).astype(np.float32)), res


def kernel(**inputs):
    y, _ = run(inputs, trace=False)
    return y


if __name__ == "__main__":
    rng = np.random.default_rng(0)
    ins = {
        "x": rng.standard_normal((B, D), dtype=np.float32),
        "W1": rng.standard_normal((E, D, H), dtype=np.float32) / 32,
        "b1": rng.standard_normal((E, H), dtype=np.float32) / 32,
        "W2": rng.standard_normal((E, H), dtype=np.float32) / 32,
        "b2": rng.standard_normal((E,), dtype=np.float32) / 32,
        "Wg": rng.standard_normal((D, E), dtype=np.float32) / 32,
        "bg": rng.standard_normal((E,), dtype=np.float32) / 32,
    }
    y = kernel(**ins)
    print("ok", y.shape, y.dtype)


# revision 16
# speedup vs baseline: 1.1990x; 1.1990x over previous
"""MoE kernel for TRN2, 8 NeuronCores, data-parallel over the batch dim.

Reference computation (B=8192, D=1024, H=1024, E=16):
    weights = softmax(x @ Wg + bg, axis=1)            # [B, E]
    h       = relu(einsum('bd,edh->beh', x, W1) + b1) # [B, E, H]
    eo      = einsum('beh,eh->be', h, W2) + b2        # [B, E]
    out     = sum(eo * weights, axis=1, keepdims=True)# [B, 1]

Strategy (v2 — bf16 matmuls, col-tiled stage 2, transposed combine):
  - Shard B over 8 cores (1024 rows/core); weights replicated.
  - All heavy matmuls in bf16 (1 cycle/row on PE + fast weight load); the
    d-contraction runs in fp32 PSUM so accuracy stays ~0.3%.
  - Stage 1 per t=(e, h_tile): psum[h=128, b=512] x2 accumulated over 8
    d-tiles from resident xT tiles; ReLU+b1 via ScalarE -> hr bf16.
  - Stage 2: w2 column blocks, 4 PSUM col-groups (partitions 32j..32j+15,
    j=t%4): batches of 4 matmuls on distinct col-groups run concurrently
    in the PE array (~4x cheaper than serial); groups are summed later via
    a replicated-weight reduction.
  - Gating stays transposed end-to-end: logits^T [128, B] with gate
    weights replicated into all 4 col-groups (pad cols zero); softmax is
    exp on ScalarE (bias=bg, pad rows -40 -> 0) + one PE reduction with a
    0.25-weighted ones vector (each expert appears 4x); no transposes.
  - Combine: v = eo_psum * expw (DVE); num = ones^T @ v + (b2/4)^T @ expw;
    y = num * reciprocal(sumexp); y^T DMA'd out as a [1, B] row.
"""

import numpy as np
import ml_dtypes

import concourse.bacc as bacc
import concourse.bass as bass
import concourse.mybir as mybir
from concourse import tile
from concourse.bass_utils import run_bass_kernel_spmd

B, D, H, E = 8192, 1024, 1024, 16
N_CORES = 8
BS = B // N_CORES  # 1024 batch rows per core
BH = 512           # psum-bank-sized half of the batch
DT = D // 128      # 8 d-tiles
HT = H // 128      # 8 h-tiles
T = E * HT         # 128 (e, h_tile) pairs
GB = 4             # stage-2 col-groups

F32 = mybir.dt.float32
F32R = mybir.dt.float32r
BF16 = mybir.dt.bfloat16
AF = mybir.ActivationFunctionType


def build_bass():
    nc = bacc.Bacc("TRN2", target_bir_lowering=False, debug=False)
    xt_d = nc.dram_tensor("xt", [D, BS], BF16, kind="ExternalInput")
    w1_d = nc.dram_tensor("w1p", [T, 128, DT * 128], BF16, kind="ExternalInput")
    b1t_d = nc.dram_tensor("b1t", [128, T], F32, kind="ExternalInput")
    w2bd_d = nc.dram_tensor("w2bd", [128, T * E], BF16, kind="ExternalInput")
    wg4_d = nc.dram_tensor("wg4", [128, DT * 128], BF16, kind="ExternalInput")
    bg4_d = nc.dram_tensor("bg4", [128, 1], F32, kind="ExternalInput")
    b2q4_d = nc.dram_tensor("b2q4", [128, 1], BF16, kind="ExternalInput")
    ones1_d = nc.dram_tensor("ones1", [128, 1], BF16, kind="ExternalInput")
    o025_d = nc.dram_tensor("o025", [128, 1], BF16, kind="ExternalInput")
    y_d = nc.dram_tensor("y", [1, BS], F32, kind="ExternalOutput")

    with tile.TileContext(nc) as tc:
        with (
            tc.tile_pool(name="const", bufs=1) as cpool,
            tc.tile_pool(name="w1", bufs=4) as w1pool,
            tc.tile_pool(name="hrelu", bufs=18) as hpool,
            tc.tile_pool(name="misc", bufs=1) as mpool,
            tc.tile_pool(name="ps1", bufs=2, space=bass.MemorySpace.PSUM) as psh,
            tc.tile_pool(name="ps_eo", bufs=1, space=bass.MemorySpace.PSUM) as pseo,
            tc.tile_pool(name="ps_aux", bufs=2, space=bass.MemorySpace.PSUM) as psaux,
        ):
            # ---- resident tensors ----
            # DMAs are queued per issuing engine: xt + consts ride the sync
            # queue, gate/W1/W2 weights ride the gpsimd queue so the first
            # weight tiles land while xt is still streaming.
            wg4_sb = cpool.tile([128, DT * 128], BF16, tag="wg4")
            nc.sync.dma_start(wg4_sb[:], wg4_d[:])
            xt_sb = []
            for d in range(DT):
                tl = cpool.tile([128, BS], BF16, tag=f"xt{d}")
                nc.sync.dma_start(tl[:], xt_d[d * 128:(d + 1) * 128, :])
                xt_sb.append(tl)
            bg4_sb = cpool.tile([128, 1], F32, tag="bg4")
            nc.sync.dma_start(bg4_sb[:], bg4_d[:])
            b1t_sb = cpool.tile([128, T], F32, tag="b1t")
            nc.sync.dma_start(b1t_sb[:], b1t_d[:])
            b2q4_sb = cpool.tile([128, 1], BF16, tag="b2q4")
            nc.sync.dma_start(b2q4_sb[:], b2q4_d[:])
            ones1_sb = cpool.tile([128, 1], BF16, tag="ones1")
            nc.sync.dma_start(ones1_sb[:], ones1_d[:])
            o025_sb = cpool.tile([128, 1], BF16, tag="o025")
            nc.sync.dma_start(o025_sb[:], o025_d[:])
            w2bd_sb = cpool.tile([128, T * E], BF16, tag="w2bd")

            expw_sb = mpool.tile([128, BS], BF16, tag="expw")
            v_sb = mpool.tile([128, BS], BF16, tag="v")
            serec_sb = mpool.tile([1, BS], F32, tag="serec")
            y_sb = mpool.tile([1, BS], F32, tag="ysb")

            # ---- gating: logits^T, replicated into the 4 col-groups ----
            glog = psaux.tile([128, BS], F32, tag="aux")
            for d in range(DT):
                for bh in range(2):
                    nc.tensor.matmul(
                        glog[:, bh * BH:(bh + 1) * BH],
                        wg4_sb[:, d * 128:(d + 1) * 128],
                        xt_sb[d][:, bh * BH:(bh + 1) * BH],
                        start=(d == 0), stop=(d == DT - 1),
                        skip_group_check=True,
                    )
            # expw = exp(logits + bg); pad rows get bias -40 -> ~0
            nc.scalar.activation(expw_sb[:], glog[:], AF.Exp, bias=bg4_sb[:])

            # ---- stage-2 accumulator; zero pad rows once ----
            eo_ps = pseo.tile([128, BS], F32)
            nc.vector.memset(eo_ps[:], 0.0)

            hrs = {}

            def flush(ts, after_bh=None):
                for bh in range(2):
                    for tt in ts:
                        g = tt % GB
                        nc.tensor.matmul(
                            eo_ps[32 * g:32 * g + 16, bh * BH:(bh + 1) * BH],
                            w2bd_sb[:, tt * E:(tt + 1) * E],
                            hrs[tt][:, bh * BH:(bh + 1) * BH],
                            start=(tt < GB), stop=(tt >= T - GB),
                            skip_group_check=True,
                            tile_position=(0, 32 * g),
                        )
                    if after_bh is not None:
                        after_bh(bh)
                for tt in ts:
                    del hrs[tt]

            # ---- main loop over t=(e, h_tile) ----
            # Stage-2 is flushed in 8-t batches, one full batch behind, so
            # the PE never waits on a freshly produced ReLU tile and the
            # full-array <-> col-tiled pipeline bubble is paid 16x, not 32x.
            FB = 2 * GB
            for t in range(T):
                if t % FB == 0 and t >= 2 * FB:
                    flush(range(t - 2 * FB, t - FB))
                w1t = w1pool.tile([128, DT * 128], BF16, tag="w1t")
                nc.scalar.dma_start(w1t[:], w1_d[t, :, :])
                if t == 2:
                    nc.scalar.dma_start(w2bd_sb[:], w2bd_d[:])
                hr = hpool.tile([128, BS], BF16, tag="hr")
                for bh in range(2):
                    ps1 = psh.tile([128, BH], F32, tag="ps1")
                    for d in range(DT):
                        nc.tensor.matmul(
                            ps1[:],
                            w1t[:, d * 128:(d + 1) * 128],
                            xt_sb[d][:, bh * BH:(bh + 1) * BH],
                            start=(d == 0), stop=(d == DT - 1),
                            skip_group_check=True,
                        )
                    nc.scalar.activation(
                        hr[:, bh * BH:(bh + 1) * BH], ps1[:], AF.Relu,
                        bias=b1t_sb[:, t:t + 1],
                    )
                hrs[t] = hr
                if t == 2:
                    # sum of gate weights (each expert appears 4x -> 0.25)
                    sumexp = psaux.tile([1, BS], F32, tag="aux")
                    for bh in range(2):
                        nc.tensor.matmul(
                            sumexp[:, bh * BH:(bh + 1) * BH],
                            o025_sb[:], expw_sb[:, bh * BH:(bh + 1) * BH],
                            start=True, stop=True, skip_group_check=True,
                        )
                    nc.vector.reciprocal(serec_sb[:], sumexp[:])
            flush(range(T - 2 * FB, T - FB))

            # ---- combine: y = (1^T(eo*expw) + (b2/4)^T expw) / sumexp ----
            # v for each batch half is computed on DVE as soon as that half's
            # last stage-2 matmul retires, overlapping the other half.
            def emit_v(bh):
                sl = slice(bh * BH, (bh + 1) * BH)
                nc.vector.tensor_mul(v_sb[:, sl], eo_ps[:, sl], expw_sb[:, sl])

            flush(range(T - FB, T), after_bh=emit_v)
            num = psaux.tile([1, BS], F32, tag="aux")
            for bh in range(2):
                sl = slice(bh * BH, (bh + 1) * BH)
                nc.tensor.matmul(
                    num[:, sl], ones1_sb[:], v_sb[:, sl],
                    start=True, stop=False, skip_group_check=True,
                )
                nc.tensor.matmul(
                    num[:, sl], b2q4_sb[:], expw_sb[:, sl],
                    start=False, stop=True, skip_group_check=True,
                )
            nc.vector.tensor_mul(y_sb[:], num[:], serec_sb[:])
            nc.sync.dma_start(y_d[:], y_sb[:])
    nc.compile()
    return nc


def prep_inputs(x, W1, b1, W2, b2, Wg, bg):
    """Host-side data prep. Returns (shared_map, per_core_xt)."""
    f = np.float32
    bf = ml_dtypes.bfloat16
    # W1 [E, D, H] -> [t=(e,ht), d_in, (d_t, h_in)]: per t one contiguous
    # block whose SBUF layout is [128 d_in, 8 d_t * 128 h]
    w1p = np.ascontiguousarray(
        np.asarray(W1, f).reshape(E, DT, 128, HT, 128)
        .transpose(0, 3, 2, 1, 4).reshape(T, 128, DT * 128)).astype(bf)
    b1t = np.ascontiguousarray(
        np.asarray(b1, f).reshape(E, HT, 128).transpose(2, 0, 1).reshape(128, T))
    w2bd = np.zeros((128, T, E), dtype=f)
    for t in range(T):
        e, ht = divmod(t, HT)
        w2bd[:, t, e] = W2[e, ht * 128:(ht + 1) * 128]
    w2bd = w2bd.reshape(128, T * E).astype(bf)
    # gate weights replicated into the 4 col-groups (16 used + 16 pad cols)
    wgr = np.asarray(Wg, f).reshape(DT, 128, E)
    wg4 = np.zeros((DT, 128, 128), dtype=f)
    for j in range(GB):
        wg4[:, :, 32 * j:32 * j + E] = wgr
    wg4 = np.ascontiguousarray(wg4.transpose(1, 0, 2).reshape(128, DT * 128)).astype(bf)
    lane = np.arange(128) % 32
    real = lane < E
    bg4 = np.full((128, 1), -40.0, f)
    bg4[real, 0] = np.tile(np.asarray(bg, f), GB)
    b2q4 = np.zeros((128, 1), f)
    b2q4[real, 0] = np.tile(np.asarray(b2, f) / 4.0, GB)
    b2q4 = b2q4.astype(bf)
    ones1 = np.where(real, 1.0, 0.0).astype(bf).reshape(128, 1)
    o025 = np.where(real, 0.25, 0.0).astype(bf).reshape(128, 1)
    shared = {"w1p": w1p, "b1t": b1t, "w2bd": w2bd, "wg4": wg4, "bg4": bg4,
              "b2q4": b2q4, "ones1": ones1, "o025": o025}
    xT = np.asarray(x, f).T.astype(bf)  # [D, B]
    xts = [np.ascontiguousarray(xT[:, c * BS:(c + 1) * BS]) for c in range(N_CORES)]
    return shared, xts


def run(inputs, trace=False):
    nc = build_bass()
    shared, xts = prep_inputs(**inputs)
    in_maps = [dict(shared, xt=xts[c]) for c in range(N_CORES)]
    res = run_bass_kernel_spmd(
        nc, in_maps, core_ids=list(range(N_CORES)), trace=trace
    )
    y = np.concatenate([r["y"] for r in res.results], axis=1)  # [1, B]
    return np.ascontiguousarray(y.reshape(B, 1).astype(np.float32)), res


def kernel(**inputs):
    y, _ = run(inputs, trace=False)
    return y


if __name__ == "__main__":
    rng = np.random.default_rng(0)
    ins = {
        "x": rng.standard_normal((B, D), dtype=np.float32),
        "W1": rng.standard_normal((E, D, H), dtype=np.float32) / 32,
        "b1": rng.standard_normal((E, H), dtype=np.float32) / 32,
        "W2": rng.standard_normal((E, H), dtype=np.float32) / 32,
        "b2": rng.standard_normal((E,), dtype=np.float32) / 32,
        "Wg": rng.standard_normal((D, E), dtype=np.float32) / 32,
        "bg": rng.standard_normal((E,), dtype=np.float32) / 32,
    }
    y = kernel(**ins)
    print("ok", y.shape, y.dtype)
